# revision 76
# baseline (speedup 1.0000x reference)
"""Causal multi-head self-attention (B=8, S=2048, D=384, H=4, Hd=96) on 8
Trainium2 NeuronCores.

Sharding: data-parallel over batch — each core processes one batch element,
weights replicated. No collectives needed.

Per-core algorithm (flash-style, fully SBUF-resident, no attention matrix in
HBM), bf16 matmul operands / fp32 PSUM accumulation / fp8 DoubleRow PV:
  - host passes x[b] pre-transposed as xT [384, 2048] bf16
  - QT/KT computed per head in [96, S] layout (d on partitions); PSUM->SBUF
    eviction fused with the bias add on the DVE (tensor_scalar_add)
  - V' = [ones | V_h] layout [S, 97*4] via augmented weight matrix (bias +
    ones column folded into the projection contraction); the ones column
    comes FIRST so the softmax denominator lands on PSUM partition 0; for
    the fp8 path V' is also evicted as per-k-tile-pair tiles [P, H, 2, 128]
    (ones | 96 dims | 31 zero pad -> legal DoubleRow M=128)
  - scoresT[k, q] = KT_h^T @ QT_h per 128-row k-tile and 512-col q-chunk;
    exp on ScalarE (PSUM->SBUF, scale=1/sqrt(Hd) folded in; for fp8 an
    extra bias ln(2^-6) prevents e4m3 overflow and cancels in the softmax);
    columns left of the causal diagonal are skipped in QK and exp; diagonal
    128x128 blocks zeroed post-exp by a 0/1 mask mul on DVE
  - PV: chunk 0 in bf16 (its small-support softmax rows dominate the max
    error and cannot afford fp8); chunks 1-3 exp to fp8e4m3 and use one
    fp8 DoubleRow matmul per non-diagonal k-tile pair (2x PE throughput),
    per-tile fp8 matmuls on the two diagonal pairs
  - acc row 0 = denominator: custom-DVE reciprocal reads PSUM partition 0
    directly -> partition_broadcast on Pool -> normalize mul on DVE (row 0
    becomes ~1 and is annihilated by a zero row 0 in the 97-row Wo)
  - output projection per head from the 97-row normalized tiles, summed in
    PSUM across heads, bias via a rank-1 ones matmul, fs copy on DVE, DMA
    to HBM
Scheduling: per q-chunk the (QK -> exp -> mask -> PV) stages stream across
heads with PV emitted PV_LAG stages late, and prev-chunk out-projection +
next-chunk projections paced through the stream as PE fillers, so the
in-order PE queue always holds ready work while the ACT engine runs the
exp chain back-to-back.
"""

import os
import sys

sys.path.insert(0, "/opt/trn_rl_repo")

import numpy as np

import concourse.bass as bass
import concourse.tile as tile
from concourse import bacc, mybir
from concourse.bass_utils import run_bass_kernel_spmd

N_CORES = 8
S = 2048
D = 384
H = 4
HD = 96
CH = 512          # q-chunk width (columns per matmul)
NCH = S // CH     # 4 q-chunks
P = 128           # k-tile height / partition dim
KTN = S // P      # 16 k-tiles
SCALE = 1.0 / np.sqrt(HD)

F32 = mybir.dt.float32
F8 = mybir.dt.float8e4
MM_DT = os.environ.get("ATTN_MM_DT", "bfloat16")  # bfloat16 | float32r
GRP = 2           # k-tiles per exp group (PSUM tile = GRP banks)
# fp8 DoubleRow PV for q-chunks >= 1 (chunk 0 stays bf16: its small-support
# softmax rows dominate max error and can't afford fp8 weights)
PVDR = os.environ.get("ATTN_PVDR", "1") == "1" and MM_DT == "bfloat16"
PC = 2.0 ** -6    # exp downscale so fp8 P never overflows; cancels in softmax
EXPB = float(np.log(PC))


def _np_mm_dt():
    if MM_DT == "bfloat16":
        import ml_dtypes
        return ml_dtypes.bfloat16
    return np.float32


def emit_pv(nc, acc, v_sb, h, nkt, kts, scols, pt):
    for j, kt in enumerate(kts):
        scol = scols[j]
        nc.tensor.matmul(
            acc[:, scol:CH],
            v_sb[kt][:, 97 * h:97 * h + 97],
            pt[:, CH * j + scol:CH * (j + 1)],
            start=(kt == 0), stop=(kt == nkt - 1),
            skip_group_check=True)


def build_nc(repeat=1, variant=(), loop_n=0):
    nc = bacc.Bacc("TRN2", target_bir_lowering=False, debug=False,
                   enable_asserts=False, num_devices=N_CORES)
    MF = mybir.dt.bfloat16 if MM_DT == "bfloat16" else mybir.dt.float32r

    W_COLS = 2 * D + 97 * H          # wq | wk | wvx rows 0..D-1
    C_COLS = P + S + D + 97 * H      # msk | ones | bo | wvb (row-0 vectors)
    xt_d = nc.dram_tensor("xt", [D, S], MF, kind="ExternalInput").ap()
    wqkv_d = nc.dram_tensor("wqkv", [D, W_COLS], MF, kind="ExternalInput").ap()
    wo_d = nc.dram_tensor("wo", [97, H * D], MF, kind="ExternalInput").ap()
    bqk_d = nc.dram_tensor("bqk", [HD, 2 * H], F32, kind="ExternalInput").ap()
    cpk_d = nc.dram_tensor("cpk", [P, C_COLS], MF, kind="ExternalInput").ap()
    if PVDR:
        mskw_d = nc.dram_tensor("mskw", [P, 2 * P], F8, kind="ExternalInput").ap()
    out_d = nc.dram_tensor("out", [S, D], F32, kind="ExternalOutput").ap()

    Exp = mybir.ActivationFunctionType.Exp
    mult = mybir.AluOpType.mult

    with tile.TileContext(nc) as tc:
        wpool = tc.alloc_tile_pool(name="w", bufs=1)
        xpool = tc.alloc_tile_pool(name="x", bufs=1)
        qkt_pool = tc.alloc_tile_pool(name="qkt", bufs=1)
        vpool = tc.alloc_tile_pool(name="v", bufs=1)
        ppool = tc.alloc_tile_pool(name="p", bufs=int(os.environ.get("ATTN_PPOOL", "4")))
        onpool = tc.alloc_tile_pool(name="on", bufs=2)
        rpool = tc.alloc_tile_pool(name="r", bufs=3)
        qepool = tc.alloc_tile_pool(name="qe", bufs=3)
        qkpool = tc.alloc_tile_pool(name="qkps", bufs=2, space="PSUM")
        accpool = tc.alloc_tile_pool(name="accps", bufs=4, space="PSUM")

        import contextlib
        loop_ctx = (tc.For_i(0, loop_n, 1) if loop_n
                    else contextlib.nullcontext())
        with loop_ctx:
          for _rep in range(repeat):
              # ---- load weights / constants ----
              xt_sb, wq_sb, wk_sb, wv_sb, wo_sb = [], [], [], [], []
              # weights on the Pool-hosted DMA queue, x on the SP queue, so
              # the two streams run in parallel; x lands chunk-by-chunk so
              # the first projections start after ~1/4 of the x transfer
              # two DMA queues, ordered by first use: SP carries x chunk 0
              # then Wq then the remaining x chunks; ACT carries Wk, biases,
              # V-weights, masks, Wo
              xt_sb = [xpool.tile([P, S], MF, name=f"xt{t}", tag=f"xt{t}")
                       for t in range(3)]
              for hf in range(2):
                  for t in range(3):
                      nc.sync.dma_start(
                          xt_sb[t][:, S // 2 * hf:S // 2 * (hf + 1)],
                          xt_d[P * t:P * t + P, S // 2 * hf:S // 2 * (hf + 1)])
              for t in range(3):
                  wt = wpool.tile([P, W_COLS], MF, name=f"wqkv{t}",
                                  tag=f"wqkv{t}")
                  nc.scalar.dma_start(wt[:], wqkv_d[P * t:P * t + P, :])
                  wq_sb.append(wt[:, 0:D])
                  wk_sb.append(wt[:, D:2 * D])
                  wv_sb.append(wt[:, 2 * D:W_COLS])
              bqk_sb = wpool.tile([HD, 2 * H], F32, name="bqk", tag="bqk")
              nc.scalar.dma_start(bqk_sb[:], bqk_d[:, :])
              bq_sb = bqk_sb[:, 0:H]
              bk_sb = bqk_sb[:, H:2 * H]
              cpk = wpool.tile([P, C_COLS], MF, name="cpk", tag="cpk")
              nc.scalar.dma_start(cpk[:], cpk_d[:, :])
              msk_sb = cpk[:, 0:P]
              ones = cpk[0:1, P:P + S]
              bo_sb = cpk[0:1, P + S:P + S + D]
              wvb = cpk[0:1, P + S + D:C_COLS]
              if PVDR:
                  mskw_sb = wpool.tile([P, 2 * P], F8, name="mskw", tag="mskw")
                  nc.scalar.dma_start(mskw_sb[:], mskw_d[:, :])
                  expb_sb = wpool.tile([P, 1], F32, name="expb", tag="expb")
                  nc.vector.memset(expb_sb[:], EXPB)
              wopk = wpool.tile([97, H * D], MF, name="wopk", tag="wopk")
              nc.scalar.dma_start(wopk[:], wo_d[:, :])
              for h in range(H):
                  wo_sb.append(wopk[:, D * h:D * h + D])

              # Q/K in per-head transposed layout [96, S]; V' in natural
              # layout [S, 97*H] with the ones col FIRST per head.
              # Projections are emitted per q-chunk, interleaved with that
              # chunk's attention, so exp (ACT) overlaps projection evictions
              # (DVE) and the PE never drains.
              qt_sb, kt_sb, v_sb = [], [], []
              qt8_sb, kt8_sb = [], []
              v8_sb = {}
              for h in range(H):
                  qt = qkt_pool.tile([HD, S], MF, name=f"qt{h}", tag=f"qt{h}")
                  qt_sb.append(qt)
                  kt = qkt_pool.tile([HD, S], MF, name=f"kt{h}", tag=f"kt{h}")
                  kt_sb.append(kt)

              def emit_qk_proj(ci, h, w_sb, b_sb, dst, dst8):
                  ps = accpool.tile([HD, CH], F32, name="projps", tag="acc")
                  for t in range(3):
                      nc.tensor.matmul(
                          ps[:],
                          w_sb[t][:, HD * h:HD * h + HD],
                          xt_sb[t][:, CH * ci:CH * ci + CH],
                          start=(t == 0), stop=(t == 2))
                  nc.vector.tensor_scalar_add(
                      dst[h][:, CH * ci:CH * ci + CH], ps[:], b_sb[:, h:h + 1])

              def emit_v_proj(st):
                  ps = accpool.tile([P, 97 * H], F32, name="vps", tag="acc")
                  for t in range(3):
                      nc.tensor.matmul(ps[:], xt_sb[t][:, P * st:P * st + P],
                                       wv_sb[t][:], start=(t == 0), stop=False)
                  nc.tensor.matmul(ps[:], ones[:, 0:P], wvb[:],
                                   start=False, stop=True)
                  if PVDR:
                      # fp8 pair tile [P, H, 2, 128]: per head, slab per
                      # k-tile, cols = [ones | v dims | 31 zero pad] so the
                      # DoubleRow output partition count is a legal 128
                      pr = st // 2
                      if pr not in v8_sb:
                          v8_sb[pr] = vpool.tile([P, H, 2, P], F8,
                                                 name=f"v8_{pr}", tag=f"v8_{pr}")
                          nc.vector.memset(v8_sb[pr][:, :, :, 97:P], 0.0)
                      nc.vector.tensor_copy(
                          v8_sb[pr][:, :, st % 2, 0:97],
                          ps[:].rearrange("p (h d) -> p h d", h=H))
                      if st < 4:
                          vt = vpool.tile([P, 97 * H], MF, name=f"v{st}",
                                          tag=f"v{st}")
                          nc.vector.tensor_copy(vt[:], ps[:])
                          v_sb.append(vt)
                  else:
                      vt = vpool.tile([P, 97 * H], MF, name=f"v{st}", tag=f"v{st}")
                      nc.scalar.copy(vt[:], ps[:])
                      v_sb.append(vt)

              def proj_units(ci):
                  us = []
                  for h in range(H):
                      us.append(lambda h=h, ci=ci: emit_qk_proj(
                          ci, h, wq_sb, bq_sb, qt_sb, qt8_sb if PVDR else None))
                      us.append(lambda h=h, ci=ci: emit_qk_proj(
                          ci, h, wk_sb, bk_sb, kt_sb, kt8_sb if PVDR else None))
                  for st in range(4 * ci, 4 * ci + 4):
                      us.append(lambda st=st: emit_v_proj(st))
                  return us

              # prologue: only what chunk 0's first head needs; the rest of
              # proj(0) rides the chunk-0 filler stream (ordered first)
              u0 = proj_units(0)
              if os.environ.get("ATTN_PROLOGUE", "full") == "full":
                  prologue, rest0 = u0, []
              else:
                  prologue = [u0[0], u0[1]] + u0[8:12]
                  rest0 = u0[2:8]
              for u in prologue:
                  u()

              # ---- attention: cross-head stage stream ----
              # Each stage = QK matmuls + exp + masks for one k-tile group.
              # PV for a stage is emitted PV_LAG stages later so the PE's
              # in-order queue always holds ready work while exp runs.
              # Fillers (prev-chunk out-proj, next-chunk projections) are
              # paced evenly through the stream to soak up spare PE cycles.
              PV_LAG = int(os.environ.get("ATTN_PV_LAG", "2"))

              # out rows are staged in sj pairs: two fs slabs share one tile
              # and one DMA (halves store-DMA count and tail sem overhead)
              fs_pend = [None]

              def emit_out_proj_sj(ci, on_tiles, sj):
                  st = 4 * ci + sj
                  fo = accpool.tile([P, D], F32, name="fo", tag="acc")
                  for h in range(H):
                      nc.tensor.matmul(fo[:], on_tiles[h][:, P * sj:P * sj + P],
                                       wo_sb[h][:], start=(h == 0), stop=False)
                  nc.tensor.matmul(fo[:], ones[:, 0:P], bo_sb[:],
                                   start=False, stop=True)
                  if sj % 2 == 0:
                      fs_pend[0] = onpool.tile([P, 2, D], F32, name="fs",
                                               tag="fs", bufs=3)
                  fs = fs_pend[0]
                  nc.vector.tensor_copy(fs[:, sj % 2, :], fo[:])
                  if sj % 2 == 1:
                      dst = out_d[P * (st - 1):P * (st + 1), :].rearrange(
                          "(s p) d -> p s d", s=2)
                      nc.sync.dma_start(dst, fs[:])

              pending = None  # (ci, on_tiles) awaiting output projection
              for ci in range(NCH):
                  nkt = 4 * (ci + 1)
                  groups = [list(range(g0, min(g0 + GRP, nkt)))
                            for g0 in range(0, nkt, GRP)]
                  stages = [(h, gi) for h in range(H)
                            for gi in range(len(groups))]
                  fillers = []
                  if pending is not None:
                      pci, ptiles = pending
                      for sj in range(4):
                          fillers.append(
                              lambda pci=pci, ptiles=ptiles, sj=sj:
                              emit_out_proj_sj(pci, ptiles, sj))
                      pending = None
                  if ci == 0:
                      fillers.extend(rest0)
                  if ci + 1 < NCH:
                      fillers.extend(proj_units(ci + 1))

                  accs = {}
                  on_tiles = [None] * H

                  use_f8 = PVDR and ci > 0

                  def stage_front(h, gi):
                      kts = groups[gi]
                      qk = qkpool.tile([P, GRP * CH], F32, name="qk", tag="qk")
                      scols = [max(P * kt - CH * ci, 0) for kt in kts]
                      for j, kt in enumerate(kts):
                          scol = scols[j]
                          nc.tensor.matmul(
                              qk[:, CH * j + scol:CH * (j + 1)],
                              kt_sb[h][:, P * kt:P * kt + P],
                              qt_sb[h][:, CH * ci + scol:CH * ci + CH],
                              start=True, stop=True)
                      pt = ppool.tile([P, GRP * CH], F8 if use_f8 else MF,
                                      name="pt", tag="pt")
                      runs = []
                      for j, kt in enumerate(kts):
                          s0, e0 = CH * j + scols[j], CH * (j + 1)
                          if runs and runs[-1][1] == s0:
                              runs[-1][1] = e0
                          else:
                              runs.append([s0, e0])
                      for s0, e0 in runs:
                          # fp8 path: scale exp down by PC so it can't
                          # overflow e4m3; the softmax ratio cancels it
                          if use_f8:
                              nc.scalar.activation(pt[:, s0:e0], qk[:, s0:e0],
                                                   Exp, scale=float(SCALE),
                                                   bias=expb_sb[:])
                          else:
                              nc.scalar.activation(pt[:, s0:e0], qk[:, s0:e0],
                                                   Exp, scale=float(SCALE))
                      for j, kt in enumerate(kts):
                          rt = P * kt - CH * ci
                          if rt >= 0:
                              # zero upper triangle of the diagonal block
                              nc.vector.tensor_mul(
                                  pt[:, CH * j + rt:CH * j + rt + P],
                                  pt[:, CH * j + rt:CH * j + rt + P],
                                  mskw_sb[:, P:2 * P] if use_f8 else msk_sb[:])
                      return (h, gi, kts, scols, pt)

                  norm_q = []

                  def emit_norm(h, acc):
                      # normalize via row 0 (the softmax denominator)
                      den0 = rpool.tile([1, CH], F32, name="den0", tag="den0")
                      nc.vector.reciprocal_approx_fast(out=den0[:],
                                                       in_=acc[0:1, :])
                      rb = rpool.tile([97, CH], F32, name="rb", tag="rb")
                      nc.gpsimd.partition_broadcast(rb[:], den0[:],
                                                    channels=97)
                      on = onpool.tile([97, CH], MF, name=f"on{h}",
                                       tag=f"on{h}")
                      nc.vector.tensor_tensor(on[:], acc[0:97, :], rb[:],
                                              op=mult)
                      on_tiles[h] = on

                  def stage_pv(h, gi, kts, scols, pt):
                      if gi == 0:
                          accs[h] = accpool.tile([P if use_f8 else 97, CH],
                                                 F32, name="acc", tag="acc")
                      pr = kts[0] // 2
                      if use_f8 and P * kts[0] - CH * ci < 0:
                          # full (non-diagonal) pair: one DoubleRow matmul
                          # over both k-tiles; rows 97..127 accumulate zeros
                          rhs3 = pt[:].rearrange("p (g n) -> p g n", g=GRP)
                          nc.tensor.matmul(
                              accs[h][:, :],
                              v8_sb[pr][:, h, :, :],
                              rhs3,
                              perf_mode=mybir.MatmulPerfMode.DoubleRow,
                              start=(kts[0] == 0), stop=(kts[-1] == nkt - 1),
                              skip_group_check=True)
                      elif use_f8:
                          # diagonal pair: per-tile fp8 matmuls with trim
                          for j, kt in enumerate(kts):
                              scol = scols[j]
                              nc.tensor.matmul(
                                  accs[h][0:97, scol:CH],
                                  v8_sb[pr][:, h, kt % 2, 0:97],
                                  pt[:, CH * j + scol:CH * (j + 1)],
                                  start=(kt == 0), stop=(kt == nkt - 1),
                                  skip_group_check=True)
                      else:
                          emit_pv(nc, accs[h], v_sb, h, nkt, kts, scols, pt)
                      while norm_q:
                          emit_norm(*norm_q.pop(0))
                      if gi == len(groups) - 1:
                          norm_q.append((h, accs.pop(h)))

                  pipe = []
                  nf = 0
                  for idx, (h, gi) in enumerate(stages):
                      pipe.append(stage_front(h, gi))
                      if len(pipe) > PV_LAG:
                          stage_pv(*pipe.pop(0))
                      want = (idx + 1) * len(fillers) // len(stages)
                      while nf < want:
                          fillers[nf]()
                          nf += 1
                  while pipe:
                      stage_pv(*pipe.pop(0))
                  while norm_q:
                      emit_norm(*norm_q.pop(0))
                  while nf < len(fillers):
                      fillers[nf]()
                      nf += 1
                  pending = (ci, on_tiles)
              pci, ptiles = pending
              for sj in range(4):
                  emit_out_proj_sj(pci, ptiles, sj)

        for pool in (accpool, qkpool, qepool, rpool, onpool, ppool, vpool,
                     qkt_pool, xpool, wpool):
            pool.release()

    nc.finalize()
    return nc


_NC_CACHE = None


def get_nc():
    global _NC_CACHE
    if _NC_CACHE is None:
        _NC_CACHE = build_nc()
    return _NC_CACHE


def host_prep(x, Wq, bq, Wk, bk, Wv, bv, Wo, bo):
    """Build per-core input maps (layout prep only; all FLOPs run on device)."""
    mdt = _np_mm_dt()
    x = np.ascontiguousarray(np.asarray(x, dtype=np.float32))
    Wq = np.ascontiguousarray(np.asarray(Wq, dtype=np.float32))
    Wk = np.ascontiguousarray(np.asarray(Wk, dtype=np.float32))
    Wv = np.ascontiguousarray(np.asarray(Wv, dtype=np.float32))
    Wo = np.ascontiguousarray(np.asarray(Wo, dtype=np.float32))
    bq = np.asarray(bq, dtype=np.float32)
    bk = np.asarray(bk, dtype=np.float32)
    bv = np.asarray(bv, dtype=np.float32)
    bo = np.asarray(bo, dtype=np.float32)

    # V' weights: per head [ones_col | v cols 0..95]; bias in row D
    wvx = np.zeros((D + 1, 97 * H), np.float32)
    for h in range(H):
        wvx[:D, 97 * h + 1:97 * h + 97] = Wv[:, HD * h:HD * h + HD]
        wvx[D, 97 * h + 1:97 * h + 97] = bv[HD * h:HD * h + HD]
        wvx[D, 97 * h] = 1.0

    # packed big-weight tensor: wq | wk | wvx rows 0..D-1
    wqkv = np.concatenate([Wq, Wk, wvx[:D]], axis=1)

    # Wo packed [97, H*D] with a zero row 0 per head (annihilates the
    # denominator row)
    wopk = np.zeros((97, H * D), np.float32)
    for h in range(H):
        wopk[1:97, D * h:D * h + D] = Wo[HD * h:HD * h + HD, :]

    jj = np.arange(P)[None, :]
    pp = np.arange(P)[:, None]
    msk = (jj >= pp).astype(np.float32)
    mskw = np.concatenate([np.zeros((P, P), np.float32), msk], axis=1)

    bqh = np.ascontiguousarray(bq.reshape(H, HD).T)
    bkh = np.ascontiguousarray(bk.reshape(H, HD).T)
    bqk = np.concatenate([bqh, bkh], axis=1)

    # const pack: msk | ones | bo | wvb-bias-row (row-0 vectors)
    C_COLS = P + S + D + 97 * H
    cpk = np.zeros((P, C_COLS), np.float32)
    cpk[:, 0:P] = msk
    cpk[0, P:P + S] = 1.0
    cpk[0, P + S:P + S + D] = bo
    cpk[0, P + S + D:C_COLS] = wvx[D]

    common = dict(wqkv=wqkv.astype(mdt), wo=wopk.astype(mdt),
                  bqk=bqk, cpk=cpk.astype(mdt))
    if PVDR:
        import ml_dtypes
        common["mskw"] = mskw.astype(ml_dtypes.float8_e4m3)
    return [dict(xt=np.ascontiguousarray(x[b].T).astype(mdt), **common)
            for b in range(x.shape[0])]


def kernel(**inputs):
    in_maps = host_prep(**inputs)
    nc = get_nc()
    res = run_bass_kernel_spmd(nc, in_maps, core_ids=list(range(N_CORES)))
    return np.stack([res.results[b]["out"] for b in range(N_CORES)], axis=0)


# revision 78
# speedup vs baseline: 1.1765x; 1.1765x over previous
"""Causal multi-head self-attention (B=8, S=2048, D=384, H=4, Hd=96) on 8
Trainium2 NeuronCores.

Sharding: data-parallel over batch — each core processes one batch element,
weights replicated. No collectives needed.

Per-core algorithm (flash-style, fully SBUF-resident, no attention matrix in
HBM), bf16 matmul operands / fp32 PSUM accumulation / fp8 DoubleRow PV:
  - host passes x[b] pre-transposed as xT [384, 2048] bf16
  - QT/KT computed per head in [96, S] layout (d on partitions); PSUM->SBUF
    eviction fused with the bias add on the DVE (tensor_scalar_add)
  - V' = [ones | V_h] layout [S, 97*4] via augmented weight matrix (bias +
    ones column folded into the projection contraction); the ones column
    comes FIRST so the softmax denominator lands on PSUM partition 0; for
    the fp8 path V' is also evicted as per-k-tile-pair tiles [P, H, 2, 128]
    (ones | 96 dims | 31 zero pad -> legal DoubleRow M=128)
  - scoresT[k, q] = KT_h^T @ QT_h per 128-row k-tile and 512-col q-chunk;
    exp on ScalarE (PSUM->SBUF, scale=1/sqrt(Hd) folded in; for fp8 an
    extra bias ln(2^-6) prevents e4m3 overflow and cancels in the softmax);
    columns left of the causal diagonal are skipped in QK and exp; diagonal
    128x128 blocks zeroed post-exp by a 0/1 mask mul on DVE
  - PV: chunk 0 in bf16 (its small-support softmax rows dominate the max
    error and cannot afford fp8); chunks 1-3 exp to fp8e4m3 and use one
    fp8 DoubleRow matmul per non-diagonal k-tile pair (2x PE throughput),
    per-tile fp8 matmuls on the two diagonal pairs
  - acc row 0 = denominator: custom-DVE reciprocal reads PSUM partition 0
    directly -> partition_broadcast on Pool -> normalize mul on DVE (row 0
    becomes ~1 and is annihilated by a zero row 0 in the 97-row Wo)
  - output projection per head from the 97-row normalized tiles, summed in
    PSUM across heads, bias via a rank-1 ones matmul, fs copy on DVE, DMA
    to HBM
Scheduling: per q-chunk the (QK -> exp -> mask -> PV) stages stream across
heads with PV emitted PV_LAG stages late, and prev-chunk out-projection +
next-chunk projections paced through the stream as PE fillers, so the
in-order PE queue always holds ready work while the ACT engine runs the
exp chain back-to-back.
"""

import os
import sys

sys.path.insert(0, "/opt/trn_rl_repo")

import numpy as np

import concourse.bass as bass
import concourse.tile as tile
from concourse import bacc, mybir
from concourse.bass_utils import run_bass_kernel_spmd

N_CORES = 8
S = 2048
D = 384
H = 4
HD = 96
CH = 512          # q-chunk width (columns per matmul)
NCH = S // CH     # 4 q-chunks
P = 128           # k-tile height / partition dim
KTN = S // P      # 16 k-tiles
SCALE = 1.0 / np.sqrt(HD)

F32 = mybir.dt.float32
F8 = mybir.dt.float8e4
MM_DT = os.environ.get("ATTN_MM_DT", "bfloat16")  # bfloat16 | float32r
GRP = 2           # k-tiles per exp group (PSUM tile = GRP banks)
# fp8 DoubleRow PV for q-chunks >= 1 (chunk 0 stays bf16: its small-support
# softmax rows dominate max error and can't afford fp8 weights)
PVDR = os.environ.get("ATTN_PVDR", "1") == "1" and MM_DT == "bfloat16"
PC = 2.0 ** -6    # exp downscale so fp8 P never overflows; cancels in softmax
EXPB = float(np.log(PC))


def _np_mm_dt():
    if MM_DT == "bfloat16":
        import ml_dtypes
        return ml_dtypes.bfloat16
    return np.float32


def emit_pv(nc, acc, v_sb, h, nkt, kts, scols, pt):
    for j, kt in enumerate(kts):
        scol = scols[j]
        nc.tensor.matmul(
            acc[:, scol:CH],
            v_sb[kt][:, 97 * h:97 * h + 97],
            pt[:, CH * j + scol:CH * (j + 1)],
            start=(kt == 0), stop=(kt == nkt - 1),
            skip_group_check=True)


def build_nc(repeat=1, variant=(), loop_n=0):
    nc = bacc.Bacc("TRN2", target_bir_lowering=False, debug=False,
                   enable_asserts=False, num_devices=N_CORES)
    MF = mybir.dt.bfloat16 if MM_DT == "bfloat16" else mybir.dt.float32r

    W_COLS = 2 * D + 97 * H          # wq | wk | wvx rows 0..D-1
    C_COLS = P + S + D + 97 * H      # msk | ones | bo | wvb (row-0 vectors)
    xt_d = nc.dram_tensor("xt", [D, S], MF, kind="ExternalInput").ap()
    wqkv_d = nc.dram_tensor("wqkv", [D, W_COLS], MF, kind="ExternalInput").ap()
    wo_d = nc.dram_tensor("wo", [97, H * D], MF, kind="ExternalInput").ap()
    bqk_d = nc.dram_tensor("bqk", [HD, 2 * H], F32, kind="ExternalInput").ap()
    cpk_d = nc.dram_tensor("cpk", [P, C_COLS], MF, kind="ExternalInput").ap()
    if PVDR:
        mskw_d = nc.dram_tensor("mskw", [P, 2 * P], F8, kind="ExternalInput").ap()
    out_d = nc.dram_tensor("out", [S, D], F32, kind="ExternalOutput").ap()

    Exp = mybir.ActivationFunctionType.Exp
    mult = mybir.AluOpType.mult

    with tile.TileContext(nc) as tc:
        wpool = tc.alloc_tile_pool(name="w", bufs=1)
        xpool = tc.alloc_tile_pool(name="x", bufs=1)
        qkt_pool = tc.alloc_tile_pool(name="qkt", bufs=1)
        vpool = tc.alloc_tile_pool(name="v", bufs=1)
        ppool = tc.alloc_tile_pool(name="p", bufs=int(os.environ.get("ATTN_PPOOL", "4")))
        onpool = tc.alloc_tile_pool(name="on", bufs=2)
        rpool = tc.alloc_tile_pool(name="r", bufs=3)
        qepool = tc.alloc_tile_pool(name="qe", bufs=3)
        qkpool = tc.alloc_tile_pool(name="qkps", bufs=2, space="PSUM")
        accpool = tc.alloc_tile_pool(name="accps", bufs=4, space="PSUM")

        import contextlib
        loop_ctx = (tc.For_i(0, loop_n, 1) if loop_n
                    else contextlib.nullcontext())
        with loop_ctx:
          for _rep in range(repeat):
              # ---- load weights / constants ----
              xt_sb, wq_sb, wk_sb, wv_sb, wo_sb = [], [], [], [], []
              # weights on the Pool-hosted DMA queue, x on the SP queue, so
              # the two streams run in parallel; x lands chunk-by-chunk so
              # the first projections start after ~1/4 of the x transfer
              # two DMA queues, ordered by first use: SP carries x chunk 0
              # then Wq then the remaining x chunks; ACT carries Wk, biases,
              # V-weights, masks, Wo
              xt_sb = [xpool.tile([P, S], MF, name=f"xt{t}", tag=f"xt{t}")
                       for t in range(3)]
              for hf in range(2):
                  for t in range(3):
                      nc.sync.dma_start(
                          xt_sb[t][:, S // 2 * hf:S // 2 * (hf + 1)],
                          xt_d[P * t:P * t + P, S // 2 * hf:S // 2 * (hf + 1)])
              for t in range(3):
                  wt = wpool.tile([P, W_COLS], MF, name=f"wqkv{t}",
                                  tag=f"wqkv{t}")
                  nc.scalar.dma_start(wt[:], wqkv_d[P * t:P * t + P, :])
                  wq_sb.append(wt[:, 0:D])
                  wk_sb.append(wt[:, D:2 * D])
                  wv_sb.append(wt[:, 2 * D:W_COLS])
              bqk_sb = wpool.tile([HD, 2 * H], F32, name="bqk", tag="bqk")
              nc.scalar.dma_start(bqk_sb[:], bqk_d[:, :])
              bq_sb = bqk_sb[:, 0:H]
              bk_sb = bqk_sb[:, H:2 * H]
              cpk = wpool.tile([P, C_COLS], MF, name="cpk", tag="cpk")
              nc.scalar.dma_start(cpk[:], cpk_d[:, :])
              msk_sb = cpk[:, 0:P]
              ones = cpk[0:1, P:P + S]
              bo_sb = cpk[0:1, P + S:P + S + D]
              wvb = cpk[0:1, P + S + D:C_COLS]
              if PVDR:
                  mskw_sb = wpool.tile([P, 2 * P], F8, name="mskw", tag="mskw")
                  nc.scalar.dma_start(mskw_sb[:], mskw_d[:, :])
                  expb_sb = wpool.tile([P, 1], F32, name="expb", tag="expb")
                  nc.vector.memset(expb_sb[:], EXPB)
              wopk = wpool.tile([97, H * D], MF, name="wopk", tag="wopk")
              nc.scalar.dma_start(wopk[:], wo_d[:, :])
              for h in range(H):
                  wo_sb.append(wopk[:, D * h:D * h + D])

              # Q/K in per-head transposed layout [96, S]; V' in natural
              # layout [S, 97*H] with the ones col FIRST per head.
              # Projections are emitted per q-chunk, interleaved with that
              # chunk's attention, so exp (ACT) overlaps projection evictions
              # (DVE) and the PE never drains.
              qt_sb, kt_sb, v_sb = [], [], []
              qt8_sb, kt8_sb = [], []
              v8_sb = {}
              for h in range(H):
                  qt = qkt_pool.tile([HD, S], MF, name=f"qt{h}", tag=f"qt{h}")
                  qt_sb.append(qt)
                  kt = qkt_pool.tile([HD, S], MF, name=f"kt{h}", tag=f"kt{h}")
                  kt_sb.append(kt)

              def emit_qk_proj(ci, h, w_sb, b_sb, dst, dst8):
                  ps = accpool.tile([HD, CH], F32, name="projps", tag="acc")
                  for t in range(3):
                      nc.tensor.matmul(
                          ps[:],
                          w_sb[t][:, HD * h:HD * h + HD],
                          xt_sb[t][:, CH * ci:CH * ci + CH],
                          start=(t == 0), stop=(t == 2))
                  nc.vector.tensor_scalar_add(
                      dst[h][:, CH * ci:CH * ci + CH], ps[:], b_sb[:, h:h + 1])

              def emit_v_proj(st):
                  ps = accpool.tile([P, 97 * H], F32, name="vps", tag="acc")
                  for t in range(3):
                      nc.tensor.matmul(ps[:], xt_sb[t][:, P * st:P * st + P],
                                       wv_sb[t][:], start=(t == 0), stop=False)
                  nc.tensor.matmul(ps[:], ones[:, 0:P], wvb[:],
                                   start=False, stop=True)
                  if PVDR:
                      # fp8 pair tile [P, H, 2, 128]: per head, slab per
                      # k-tile, cols = [ones | v dims | 31 zero pad] so the
                      # DoubleRow output partition count is a legal 128
                      pr = st // 2
                      if pr not in v8_sb:
                          v8_sb[pr] = vpool.tile([P, H, 2, P], F8,
                                                 name=f"v8_{pr}", tag=f"v8_{pr}")
                          nc.vector.memset(v8_sb[pr][:, :, :, 97:P], 0.0)
                      nc.vector.tensor_copy(
                          v8_sb[pr][:, :, st % 2, 0:97],
                          ps[:].rearrange("p (h d) -> p h d", h=H))
                      if st < 4:
                          vt = vpool.tile([P, 97 * H], MF, name=f"v{st}",
                                          tag=f"v{st}")
                          nc.vector.tensor_copy(vt[:], ps[:])
                          v_sb.append(vt)
                  else:
                      vt = vpool.tile([P, 97 * H], MF, name=f"v{st}", tag=f"v{st}")
                      nc.scalar.copy(vt[:], ps[:])
                      v_sb.append(vt)

              def proj_units(ci):
                  us = []
                  for h in range(H):
                      us.append(lambda h=h, ci=ci: emit_qk_proj(
                          ci, h, wq_sb, bq_sb, qt_sb, qt8_sb if PVDR else None))
                      us.append(lambda h=h, ci=ci: emit_qk_proj(
                          ci, h, wk_sb, bk_sb, kt_sb, kt8_sb if PVDR else None))
                  for st in range(4 * ci, 4 * ci + 4):
                      us.append(lambda st=st: emit_v_proj(st))
                  return us

              # prologue: only what chunk 0's first head needs; the rest of
              # proj(0) rides the chunk-0 filler stream (ordered first)
              u0 = proj_units(0)
              if os.environ.get("ATTN_PROLOGUE", "full") == "full":
                  prologue, rest0 = u0, []
              else:
                  prologue = [u0[0], u0[1]] + u0[8:12]
                  rest0 = u0[2:8]
              for u in prologue:
                  u()

              # ---- attention: cross-head stage stream ----
              # Each stage = QK matmuls + exp + masks for one k-tile group.
              # PV for a stage is emitted PV_LAG stages later so the PE's
              # in-order queue always holds ready work while exp runs.
              # Fillers (prev-chunk out-proj, next-chunk projections) are
              # paced evenly through the stream to soak up spare PE cycles.
              PV_LAG = int(os.environ.get("ATTN_PV_LAG", "2"))

              # out rows are staged in sj pairs: two fs slabs share one tile
              # and one DMA (halves store-DMA count and tail sem overhead)
              fs_pend = [None]

              def emit_out_proj_sj(ci, on_tiles, sj):
                  st = 4 * ci + sj
                  fo = accpool.tile([P, D], F32, name="fo", tag="acc")
                  for h in range(H):
                      nc.tensor.matmul(fo[:], on_tiles[h][:, P * sj:P * sj + P],
                                       wo_sb[h][:], start=(h == 0), stop=False)
                  nc.tensor.matmul(fo[:], ones[:, 0:P], bo_sb[:],
                                   start=False, stop=True)
                  if sj % 2 == 0:
                      fs_pend[0] = onpool.tile([P, 2, D], F32, name="fs",
                                               tag="fs", bufs=3)
                  fs = fs_pend[0]
                  nc.vector.tensor_copy(fs[:, sj % 2, :], fo[:])
                  if sj % 2 == 1:
                      dst = out_d[P * (st - 1):P * (st + 1), :].rearrange(
                          "(s p) d -> p s d", s=2)
                      nc.sync.dma_start(dst, fs[:])

              pending = None  # (ci, on_tiles) awaiting output projection
              for ci in range(NCH):
                  nkt = 4 * (ci + 1)
                  groups = [list(range(g0, min(g0 + GRP, nkt)))
                            for g0 in range(0, nkt, GRP)]
                  stages = [(h, gi) for h in range(H)
                            for gi in range(len(groups))]
                  fillers = []
                  if pending is not None:
                      pci, ptiles = pending
                      for sj in range(4):
                          fillers.append(
                              lambda pci=pci, ptiles=ptiles, sj=sj:
                              emit_out_proj_sj(pci, ptiles, sj))
                      pending = None
                  if ci == 0:
                      fillers.extend(rest0)
                  if ci + 1 < NCH:
                      fillers.extend(proj_units(ci + 1))

                  accs = {}
                  on_tiles = [None] * H

                  use_f8 = PVDR and ci > 0

                  def stage_front(h, gi):
                      kts = groups[gi]
                      qk = qkpool.tile([P, GRP * CH], F32, name="qk", tag="qk")
                      scols = [max(P * kt - CH * ci, 0) for kt in kts]
                      for j, kt in enumerate(kts):
                          scol = scols[j]
                          nc.tensor.matmul(
                              qk[:, CH * j + scol:CH * (j + 1)],
                              kt_sb[h][:, P * kt:P * kt + P],
                              qt_sb[h][:, CH * ci + scol:CH * ci + CH],
                              start=True, stop=True)
                      pt = ppool.tile([P, GRP * CH], F8 if use_f8 else MF,
                                      name="pt", tag="pt")
                      runs = []
                      for j, kt in enumerate(kts):
                          s0, e0 = CH * j + scols[j], CH * (j + 1)
                          if runs and runs[-1][1] == s0:
                              runs[-1][1] = e0
                          else:
                              runs.append([s0, e0])
                      for s0, e0 in runs:
                          # fp8 path: scale exp down by PC so it can't
                          # overflow e4m3; the softmax ratio cancels it
                          if use_f8:
                              nc.scalar.activation(pt[:, s0:e0], qk[:, s0:e0],
                                                   Exp, scale=float(SCALE),
                                                   bias=expb_sb[:])
                          else:
                              nc.scalar.activation(pt[:, s0:e0], qk[:, s0:e0],
                                                   Exp, scale=float(SCALE))
                      for j, kt in enumerate(kts):
                          rt = P * kt - CH * ci
                          if rt >= 0:
                              # zero upper triangle of the diagonal block
                              nc.vector.tensor_mul(
                                  pt[:, CH * j + rt:CH * j + rt + P],
                                  pt[:, CH * j + rt:CH * j + rt + P],
                                  mskw_sb[:, P:2 * P] if use_f8 else msk_sb[:])
                      return (h, gi, kts, scols, pt)

                  norm_q = []

                  def emit_norm(h, acc):
                      # normalize via row 0 (the softmax denominator)
                      den0 = rpool.tile([1, CH], F32, name="den0", tag="den0")
                      nc.vector.reciprocal_approx_fast(out=den0[:],
                                                       in_=acc[0:1, :])
                      rb = rpool.tile([97, CH], F32, name="rb", tag="rb")
                      nc.gpsimd.partition_broadcast(rb[:], den0[:],
                                                    channels=97)
                      on = onpool.tile([97, CH], MF, name=f"on{h}",
                                       tag=f"on{h}")
                      nc.vector.tensor_tensor(on[:], acc[0:97, :], rb[:],
                                              op=mult)
                      on_tiles[h] = on

                  def stage_pv(h, gi, kts, scols, pt):
                      if gi == 0:
                          accs[h] = accpool.tile([P if use_f8 else 97, CH],
                                                 F32, name="acc", tag="acc")
                      pr = kts[0] // 2
                      if use_f8 and P * kts[0] - CH * ci < 0:
                          # full (non-diagonal) pair: one DoubleRow matmul
                          # over both k-tiles; rows 97..127 accumulate zeros
                          rhs3 = pt[:].rearrange("p (g n) -> p g n", g=GRP)
                          nc.tensor.matmul(
                              accs[h][:, :],
                              v8_sb[pr][:, h, :, :],
                              rhs3,
                              perf_mode=mybir.MatmulPerfMode.DoubleRow,
                              start=(kts[0] == 0), stop=(kts[-1] == nkt - 1),
                              skip_group_check=True)
                      elif use_f8:
                          # diagonal pair: per-tile fp8 matmuls with trim
                          for j, kt in enumerate(kts):
                              scol = scols[j]
                              nc.tensor.matmul(
                                  accs[h][0:97, scol:CH],
                                  v8_sb[pr][:, h, kt % 2, 0:97],
                                  pt[:, CH * j + scol:CH * (j + 1)],
                                  start=(kt == 0), stop=(kt == nkt - 1),
                                  skip_group_check=True)
                      else:
                          emit_pv(nc, accs[h], v_sb, h, nkt, kts, scols, pt)
                      while norm_q:
                          emit_norm(*norm_q.pop(0))
                      if gi == len(groups) - 1:
                          norm_q.append((h, accs.pop(h)))

                  pipe = []
                  nf = 0
                  for idx, (h, gi) in enumerate(stages):
                      pipe.append(stage_front(h, gi))
                      if len(pipe) > PV_LAG:
                          stage_pv(*pipe.pop(0))
                      want = (idx + 1) * len(fillers) // len(stages)
                      while nf < want:
                          fillers[nf]()
                          nf += 1
                  while pipe:
                      stage_pv(*pipe.pop(0))
                  while norm_q:
                      emit_norm(*norm_q.pop(0))
                  while nf < len(fillers):
                      fillers[nf]()
                      nf += 1
                  pending = (ci, on_tiles)
              pci, ptiles = pending
              for sj in range(4):
                  emit_out_proj_sj(pci, ptiles, sj)

        for pool in (accpool, qkpool, qepool, rpool, onpool, ppool, vpool,
                     qkt_pool, xpool, wpool):
            pool.release()

    nc.finalize()
    return nc


_NC_CACHE = None


def get_nc():
    global _NC_CACHE
    if _NC_CACHE is None:
        _NC_CACHE = build_nc()
    return _NC_CACHE


def host_prep(x, Wq, bq, Wk, bk, Wv, bv, Wo, bo):
    """Build per-core input maps (layout prep only; all FLOPs run on device)."""
    mdt = _np_mm_dt()
    x = np.ascontiguousarray(np.asarray(x, dtype=np.float32))
    Wq = np.ascontiguousarray(np.asarray(Wq, dtype=np.float32))
    Wk = np.ascontiguousarray(np.asarray(Wk, dtype=np.float32))
    Wv = np.ascontiguousarray(np.asarray(Wv, dtype=np.float32))
    Wo = np.ascontiguousarray(np.asarray(Wo, dtype=np.float32))
    bq = np.asarray(bq, dtype=np.float32)
    bk = np.asarray(bk, dtype=np.float32)
    bv = np.asarray(bv, dtype=np.float32)
    bo = np.asarray(bo, dtype=np.float32)

    # V' weights: per head [ones_col | v cols 0..95]; bias in row D
    wvx = np.zeros((D + 1, 97 * H), np.float32)
    for h in range(H):
        wvx[:D, 97 * h + 1:97 * h + 97] = Wv[:, HD * h:HD * h + HD]
        wvx[D, 97 * h + 1:97 * h + 97] = bv[HD * h:HD * h + HD]
        wvx[D, 97 * h] = 1.0

    # packed big-weight tensor: wq | wk | wvx rows 0..D-1
    wqkv = np.concatenate([Wq, Wk, wvx[:D]], axis=1)

    # Wo packed [97, H*D] with a zero row 0 per head (annihilates the
    # denominator row)
    wopk = np.zeros((97, H * D), np.float32)
    for h in range(H):
        wopk[1:97, D * h:D * h + D] = Wo[HD * h:HD * h + HD, :]

    jj = np.arange(P)[None, :]
    pp = np.arange(P)[:, None]
    msk = (jj >= pp).astype(np.float32)
    mskw = np.concatenate([np.zeros((P, P), np.float32), msk], axis=1)

    bqh = np.ascontiguousarray(bq.reshape(H, HD).T)
    bkh = np.ascontiguousarray(bk.reshape(H, HD).T)
    bqk = np.concatenate([bqh, bkh], axis=1)

    # const pack: msk | ones | bo | wvb-bias-row (row-0 vectors)
    C_COLS = P + S + D + 97 * H
    cpk = np.zeros((P, C_COLS), np.float32)
    cpk[:, 0:P] = msk
    cpk[0, P:P + S] = 1.0
    cpk[0, P + S:P + S + D] = bo
    cpk[0, P + S + D:C_COLS] = wvx[D]

    common = dict(wqkv=wqkv.astype(mdt), wo=wopk.astype(mdt),
                  bqk=bqk, cpk=cpk.astype(mdt))
    if PVDR:
        import ml_dtypes
        common["mskw"] = mskw.astype(ml_dtypes.float8_e4m3)
    return [dict(xt=np.ascontiguousarray(x[b].T).astype(mdt), **common)
            for b in range(x.shape[0])]


def kernel(**inputs):
    in_maps = host_prep(**inputs)
    nc = get_nc()
    res = run_bass_kernel_spmd(nc, in_maps, core_ids=list(range(N_CORES)))
    return np.stack([res.results[b]["out"] for b in range(N_CORES)], axis=0)
